# revision 10
# baseline (speedup 1.0000x reference)
"""Trainium2 Bass kernel for the DIN-style pairwise-interaction attention module.

Math (per batch b):
  h = x @ ln_w + ln_b                                  [L, H]
  pre[i,j,a] = a_j + c_i + cross_ij + b1[a]            (w1a/w1b/w1c split of w1)
  score[i,j] = sum_a w2[a]*leaky_relu(pre) + b2, causal-masked (j<=i)
  out = score @ h

Strategy: data-parallel over B=32 across 8 cores (4 batches/core).
Per (b, a-channel): psum[j,i] = s_a * pre  built by two accumulating matmuls:
  MM1: lhsT=[hT;ones] (shared), rhs_a=[s_a*w1c_a . hT ; s_a*w1b_a . hT]  -> cross + c_i
  MM2: lhsT=[aT';ones] (shared), rhs=one-hot host constant               -> a_j + b1
All channels scaled by s_a=|w2[a]| (lrelu positive homogeneity); channels
permuted pos-first and the w2<0 block is SUBTRACTED after separate fold-trees
(HW Lrelu has fixed 0.01 slope; its alpha operand is ignored).
Causal split j in [0,128),[128,200) limits i-extent to 200/72.
"""

import os
import sys

import numpy as np

if "/opt/trn_rl_repo" not in sys.path:
    sys.path.insert(0, "/opt/trn_rl_repo")

import ml_dtypes  # noqa: E402

BF = ml_dtypes.bfloat16

B, L, D = 32, 200, 64
H, A = 64, 36
NEG_SLOPE = 0.01
NCORES = 8
BPC = B // NCORES  # batches per core
J0, J1 = 128, 72  # causal j-blocks: [0,128) with i in [0,200); [128,200) with i in [128,200)
ALPHA_NEG = 1.0 / NEG_SLOPE


def _host_prep(ln_w, ln_b, w1, b1, w2, b2):
    """Permute channels (w2>=0 first) and fold per-channel scales into weights."""
    w1a, w1b, w1c = w1[:H], w1[H : 2 * H], w1[2 * H :]
    pos = w2 >= 0
    perm = np.concatenate([np.where(pos)[0], np.where(~pos)[0]])
    npos = int(pos.sum())
    w1a, w1b, w1c = w1a[:, perm], w1b[:, perm], w1c[:, perm]
    b1p, w2p = b1[perm], w2[perm]
    s = np.abs(w2p).astype(np.float32)  # [A]; sign handled by subtract-fold

    scl = np.vstack([w1c * s, w1b * s]).astype(np.float32)  # [128, A]
    w1as = np.zeros((D + 1, A + 1), np.float32)  # [65, 37]: aT' cols + ones col
    w1as[0:D, 0:A] = w1a * s
    w1as[D, A] = 1.0  # row A of aT-psum = ones (reads lhs1's ones row)
    w1as = w1as.astype(BF)
    lnw = np.vstack([ln_w, ln_b[None, :]]).astype(BF)  # [D+1, H]
    b1s = (b1p * s).astype(np.float32)
    oh = np.zeros((A + 1, A * L), dtype=np.float32)  # one-hot + b1 row
    for a in range(A):
        oh[a, a * L : (a + 1) * L] = 1.0
        oh[A, a * L : (a + 1) * L] = b1s[a]
    oh = oh.astype(BF)
    idm = np.eye(128, dtype=BF)
    m0 = (np.arange(L)[None, :] >= np.arange(J0)[:, None]).astype(BF)  # [J0, L]
    m1 = (np.arange(J1)[None, :] >= np.arange(J1)[:, None]).astype(BF)  # [J1, J1]
    return dict(scl=scl, w1as=w1as, lnw=lnw, oh=oh, idm=idm, m0=m0, m1=m1), npos, float(b2)


def _sign_runs(c0, c1, npos):
    """Split channel range [c0,c1) into (lo,hi,alpha) runs uniform in w2-sign."""
    runs = []
    if c0 < min(c1, npos):
        runs.append((c0, min(c1, npos), NEG_SLOPE))
    if max(c0, npos) < c1:
        runs.append((max(c0, npos), c1, ALPHA_NEG))
    return runs


def _build(npos, b2):
    import concourse.bacc as bacc
    import concourse.bass as bass  # noqa: F401
    import concourse.tile as tile
    from concourse import mybir

    f32, bf16 = mybir.dt.float32, mybir.dt.bfloat16
    LR = mybir.ActivationFunctionType.Lrelu

    nc = bacc.Bacc("TRN2", target_bir_lowering=False, debug=False)
    x_d = nc.dram_tensor("x", [BPC, L, D], bf16, kind="ExternalInput")
    out_d = nc.dram_tensor("out", [BPC, L, H], f32, kind="ExternalOutput")
    scl_d = nc.dram_tensor("scl", [128, A], f32, kind="ExternalInput")
    w1as_d = nc.dram_tensor("w1as", [D + 1, A + 1], bf16, kind="ExternalInput")
    lnw_d = nc.dram_tensor("lnw", [D + 1, H], bf16, kind="ExternalInput")
    oh_d = nc.dram_tensor("oh", [A + 1, A * L], bf16, kind="ExternalInput")
    idm_d = nc.dram_tensor("idm", [128, 128], bf16, kind="ExternalInput")
    m0_d = nc.dram_tensor("m0", [J0, L], bf16, kind="ExternalInput")
    m1_d = nc.dram_tensor("m1", [J1, J1], bf16, kind="ExternalInput")

    NPAIR = A // 2  # 18 channel pairs for jb0 (2x200=400 cols per psum bank)
    PW = 3  # pairs per jb0 psum tile (3 banks)
    C1 = 12  # channels per jb1 psum tile (12 x 128-slot, 3 banks)

    with tile.TileContext(nc) as tc:
        with (
            tc.tile_pool(name="consts", bufs=1) as cp,
            tc.tile_pool(name="work", bufs=2) as wp,
            tc.tile_pool(name="psw", bufs=2, space="PSUM") as psw,
            tc.tile_pool(name="psp", bufs=2, space="PSUM") as psp,
        ):
            scl = cp.tile([128, A], f32)
            nc.sync.dma_start(scl[:], scl_d[:])
            w1as = cp.tile([D + 1, A + 1], bf16)
            nc.sync.dma_start(w1as[:], w1as_d[:])
            lnw = cp.tile([D + 1, H], bf16)
            nc.sync.dma_start(lnw[:], lnw_d[:])
            oh = cp.tile([A + 1, A * L], bf16)
            nc.sync.dma_start(oh[:], oh_d[:])
            idm = cp.tile([128, 128], bf16)
            nc.sync.dma_start(idm[:], idm_d[:])
            m0 = cp.tile([J0, L], bf16)
            nc.sync.dma_start(m0[:], m0_d[:])
            m1 = cp.tile([J1, J1], bf16)
            nc.sync.dma_start(m1[:], m1_d[:])

            for bi in range(BPC):
                # ---- load x, build xT = [x^T; ones] ----
                x0 = wp.tile([128, D], bf16, tag="x0")
                nc.sync.dma_start(x0[:], x_d[bi, 0:128, :])
                x1 = wp.tile([J1, D], bf16, tag="x1")
                nc.sync.dma_start(x1[:], x_d[bi, 128:L, :])
                xT = wp.tile([D + 1, L], bf16, tag="xT")
                pt0 = psp.tile([D, 128], bf16, tag="pp")
                nc.tensor.transpose(pt0[:], x0[:], idm[:, :])
                nc.vector.tensor_copy(xT[0:D, 0:128], pt0[:])
                pt1 = psp.tile([D, J1], bf16, tag="pp")
                nc.tensor.transpose(pt1[:], x1[:], idm[0:J1, 0:J1])
                nc.vector.tensor_copy(xT[0:D, 128:L], pt1[:])
                nc.vector.memset(xT[D : D + 1, :], 1.0)

                # ---- hT [H, L]; hh=[hT;hT]; lhs1=[hT;ones] ----
                ph = psp.tile([H, L], f32, tag="pp")
                nc.tensor.matmul(ph[:], lnw[:], xT[:], start=True, stop=True)
                hh = wp.tile([128, L], bf16, tag="hh")
                lhs1 = wp.tile([128, L], bf16, tag="lhs1")
                nc.scalar.copy(hh[0:H, :], ph[:])
                nc.scalar.copy(hh[H:128, :], ph[:])
                nc.scalar.copy(lhs1[0:H, :], ph[:])
                nc.vector.memset(lhs1[H:128, :], 1.0)

                # ---- h natural [L, H] (two row blocks), bf16 ----
                ph0 = psp.tile([128, H], f32, tag="pp")
                nc.tensor.matmul(ph0[:], xT[:, 0:128], lnw[:], start=True, stop=True)
                h0 = wp.tile([128, H], bf16, tag="h0")
                nc.scalar.copy(h0[:], ph0[:])
                ph1 = psp.tile([J1, H], f32, tag="pp")
                nc.tensor.matmul(ph1[:], xT[:, 128:L], lnw[:], start=True, stop=True)
                h1 = wp.tile([J1, H], bf16, tag="h1")
                nc.scalar.copy(h1[:], ph1[:])

                # ---- aTs = [aT'; ones], ones row via w1as selector column ----
                pa = psp.tile([A + 1, L], f32, tag="pp")
                nc.tensor.matmul(pa[:], w1as[:], lhs1[0:D + 1, :], start=True, stop=True)
                aTs = wp.tile([A + 1, L], bf16, tag="aTs")
                nc.scalar.copy(aTs[:], pa[:])

                # ---- per-channel moving operand: rhs_a = scl[:,a] * hh ----
                rhs = wp.tile([128, A * L], bf16, tag="rhs")
                for a in range(A):
                    nc.vector.tensor_scalar_mul(
                        rhs[:, a * L : (a + 1) * L], hh[:], scl[:, a : a + 1]
                    )

                r0 = wp.tile([J0, A * L], bf16, tag="r0")
                r1 = wp.tile([J1, A * J1], bf16, tag="r1")

                # ---- jb0: j in [0,128), i in [0,200); pairs of channels ----
                for p0 in range(0, NPAIR, PW):
                    pn = min(PW, NPAIR - p0)
                    pw = psw.tile([J0, PW * 512], f32, tag="pw")
                    pwv = pw[:, :].rearrange("p (g x) -> p g x", x=512)
                    for p in range(p0, p0 + pn):
                        sl = pw[:, (p - p0) * 512 : (p - p0) * 512 + 400]
                        nc.tensor.matmul(
                            sl,
                            lhs1[:, 0:J0],
                            rhs[:, p * 2 * L : (p + 1) * 2 * L],
                            start=True,
                            stop=False,
                        )
                    for p in range(p0, p0 + pn):
                        sl = pw[:, (p - p0) * 512 : (p - p0) * 512 + 400]
                        nc.tensor.matmul(
                            sl,
                            aTs[:, 0:J0],
                            oh[:, p * 2 * L : (p + 1) * 2 * L],
                            start=False,
                            stop=True,
                        )
                    nc.scalar.activation(
                        r0[:, 2 * p0 * L : 2 * (p0 + pn) * L].rearrange(
                            "p (g x) -> p g x", x=2 * L
                        ),
                        pwv[:, 0:pn, 0:400],
                        LR,
                        alpha=NEG_SLOPE,
                    )

                # ---- jb1: j in [128,200), i in [128,200); 128-padded slots ----
                rhsv = rhs[:, :].rearrange("p (c x) -> p c x", x=L)
                ohv = oh[:, :].rearrange("p (c x) -> p c x", x=L)
                for c0 in range(0, A, C1):
                    cn = min(C1, A - c0)
                    pz = psw.tile([J1, PW * 512], f32, tag="pw")
                    pzv = pz[:, :].rearrange("p (g x) -> p g x", x=128)
                    for g0 in range(0, cn, 4):
                        gn = min(4, cn - g0)
                        nc.tensor.matmul(
                            pzv[:, g0 : g0 + gn, 0:J1],
                            lhs1[:, 128:L],
                            rhsv[:, c0 + g0 : c0 + g0 + gn, 128:L],
                            start=True,
                            stop=False,
                        )
                    for g0 in range(0, cn, 4):
                        gn = min(4, cn - g0)
                        nc.tensor.matmul(
                            pzv[:, g0 : g0 + gn, 0:J1],
                            aTs[:, 128:L],
                            ohv[:, c0 + g0 : c0 + g0 + gn, 128:L],
                            start=False,
                            stop=True,
                        )
                    nc.scalar.activation(
                        r1[:, c0 * J1 : (c0 + cn) * J1].rearrange(
                            "p (g x) -> p g x", x=J1
                        ),
                        pzv[:, 0:cn, 0:J1],
                        LR,
                        alpha=NEG_SLOPE,
                    )

                # ---- channel fold-trees (pos block, neg block), then
                # score = (pos_sum + b2) - neg_sum, then mask ----
                P, N = npos, A - npos

                def fold_range(reg, c0, w, stride):
                    W = w
                    while W > 1:
                        half = W // 2
                        keep = W - half
                        nc.vector.tensor_add(
                            reg[:, c0 * stride : (c0 + half) * stride],
                            reg[:, c0 * stride : (c0 + half) * stride],
                            reg[:, (c0 + keep) * stride : (c0 + W) * stride],
                        )
                        W = keep

                sm0 = wp.tile([J0, L], bf16, tag="sm0")
                sm1 = wp.tile([J1, J1], bf16, tag="sm1")
                for reg, stride, sm in ((r0, L, sm0), (r1, J1, sm1)):
                    if P > 0:
                        fold_range(reg, 0, P, stride)
                    if N > 0:
                        fold_range(reg, P, N, stride)
                    if P > 0 and N > 0:
                        nc.vector.scalar_tensor_tensor(
                            sm[:],
                            reg[:, 0:stride],
                            b2,
                            reg[:, P * stride : (P + 1) * stride],
                            mybir.AluOpType.add,
                            mybir.AluOpType.subtract,
                        )
                    elif N == 0:
                        nc.vector.tensor_scalar_add(sm[:], reg[:, 0:stride], b2)
                    else:
                        nc.vector.tensor_scalar(
                            sm[:], reg[:, 0:stride], -1.0, b2,
                            mybir.AluOpType.mult, mybir.AluOpType.add,
                        )
                nc.vector.tensor_mul(sm0[:], sm0[:], m0[:])
                nc.vector.tensor_mul(sm1[:], sm1[:], m1[:])

                # ---- out = score^T-masked @ h ----
                po1 = psp.tile([128, H], f32, tag="pp")
                nc.tensor.matmul(po1[:], sm0[:, 0:128], h0[:], start=True, stop=True)
                po2 = psp.tile([J1, H], f32, tag="pp")
                nc.tensor.matmul(po2[:], sm0[:, 128:L], h0[:], start=True, stop=False)
                nc.tensor.matmul(po2[:], sm1[:], h1[:], start=False, stop=True)
                o0 = wp.tile([128, H], f32, tag="o0")
                nc.scalar.copy(o0[:], po1[:])
                o1 = wp.tile([J1, H], f32, tag="o1")
                nc.scalar.copy(o1[:], po2[:])
                nc.sync.dma_start(out_d[bi, 0:128, :], o0[:])
                nc.sync.dma_start(out_d[bi, 128:L, :], o1[:])

    if not nc.is_finalized():
        nc.finalize()
    return nc


_CACHE = {}


def kernel(x, ln_w, ln_b, w1, b1, w2, b2):
    from concourse.bass_utils import run_bass_kernel_spmd

    x = np.asarray(x, dtype=np.float32)
    consts, npos, b2f = _host_prep(
        np.asarray(ln_w, np.float32),
        np.asarray(ln_b, np.float32),
        np.asarray(w1, np.float32),
        np.asarray(b1, np.float32),
        np.asarray(w2, np.float32),
        np.asarray(b2, np.float32),
    )
    key = (npos, round(b2f, 9))
    if key not in _CACHE:
        _CACHE[key] = _build(npos, b2f)
    nc = _CACHE[key]

    xb = x.astype(BF)
    in_maps = []
    for c in range(NCORES):
        m = {"x": xb[c * BPC : (c + 1) * BPC]}
        m.update(consts)
        in_maps.append(m)

    trace = bool(int(os.environ.get("KERNEL_TRACE", "0")))
    res = run_bass_kernel_spmd(nc, in_maps, list(range(NCORES)), trace=trace)
    out = np.concatenate([res.results[c]["out"] for c in range(NCORES)], axis=0)
    if trace:
        kernel.last_exec_time_ns = res.exec_time_ns
        kernel.last_results = res
    return out.astype(np.float32)
